# revision 1
# baseline (speedup 1.0000x reference)
"""CropProposals (adaptive max-pool 3d over per-proposal boxes) on 8 trn2 cores.

Sharding: core k handles batch b = k//2, channel half ch = k%2 (data parallel,
no cross-core communication).  fm slice per core: [32, 24, 24, 24] f32.

Math (matches the torch/jax reference exactly):
  per proposal and axis: lo = floor(clip(c0/4, 0, 21)); n = floor(clip(...)) - lo
  AdaptiveMaxPool output_size=2 bins: bin k = [lo + (k*n)//2, lo + ((k+1)*n+1)//2)
  Both bins have EQUAL length l = ceil(n/2), starts lo and lo + n//2 -> a
  regular 2-entry strided access pattern per axis.

Device algorithm per core (single SBUF-resident volume):
  A [128 part = (d%4, c), free = (dq=d//4 :6, h :24, w :24)]   <- 6 DMAs (per dq)
  stage 1: per (prop, hb): reduce_max over (w-bins x lh x lw) keeping (dq, wb)
           -> B [128, (g, hb, wb, dq, pi)]    (g = proposal group of 16)
  shuffle: per (g, dm4) SBUF->SBUF DMA -> C [128 part = (c, hb, wb), (g, d, pi)]
  stage 2: per prop: reduce_max over d-bins -> D [128, (sp, db)]
  out: one DMA -> DRAM [128, 128]; host reassembles to [B,P,C,2,2,2].

Proposals are sorted by their max needed dq so stage 1 can start while later
input DMAs stream, and grouped into a lag-3 software pipeline so each group's
shuffle + stage 2 hide under later groups' stage 1.  Per-core programs differ
(boxes are baked into APs), so the single SPMD program carries one tc.If arm
per batch, dispatched on partition_id >> 1; arms hold only DVE reduces.
"""

import os
import sys

import numpy as np

for _p in ("/opt/trn_rl_repo", os.path.expanduser("~/.axon_site/_ro/trn_rl_repo")):
    if os.path.isdir(_p) and _p not in sys.path:
        sys.path.insert(0, _p)

import concourse.bass as bass  # noqa: E402
import concourse.tile as tile  # noqa: E402
import concourse.tile_sem_assignment as _tsa  # noqa: E402
from concourse import mybir  # noqa: E402
from concourse.bass_utils import run_bass_kernel_spmd  # noqa: E402

# This walrus build encodes at most ONE sync-wait per compute instruction
# (TensorReduce with 2 waits fails codegen with "Too many sync wait
# commands").  Tile assigns HWDGE DMA completion semaphores round-robin over
# 8 lanes, so a reduce consuming several DMAs would wait on several sems.
# Pin the lane by issuing engine instead (SP queue -> lane 0, Activation
# queue -> lane 1): every consumer then waits on a single per-queue FIFO
# semaphore, which is also completion-order-sound (one hardware queue per
# lane).
if not getattr(_tsa, "_ant_engine_keyed_lanes", False):
    _orig_assign_tick = _tsa.TileClockTick._assign_tick

    def _assign_tick_engine_lanes(self, inst):
        if isinstance(inst, _tsa.DMAInst) and inst.engine in (
            mybir.EngineType.SP,
            mybir.EngineType.Activation,
        ):
            self.next_hw_dma_idx = 0 if inst.engine == mybir.EngineType.SP else 1
        return _orig_assign_tick(self, inst)

    _tsa.TileClockTick._assign_tick = _assign_tick_engine_lanes
    _tsa._ant_engine_keyed_lanes = True

B, P, C, S = 4, 64, 64, 24
CH = C // 2   # channels per core
NG = 4        # proposal groups
GP = P // NG  # proposals per group
GSZ = 2 * 2 * 6 * GP  # B-tile elems per group


def _bin_params(corners: np.ndarray):
    """Replicate the reference's float32 bin math. Returns lo, n int64 [B,P,3]
    with axis order (d, h, w)."""
    c = corners.astype(np.float32) / np.float32(4.0)
    LL = np.clip(c[:, :, 0, :], np.float32(0.0), np.float32(21.0))
    UR = c[:, :, 1, :]
    UR = np.where(UR - LL >= np.float32(2.0), UR, LL + np.float32(2.0))
    UR = np.clip(UR, np.float32(2.0), np.float32(23.0))
    lo = np.floor(LL).astype(np.int64)
    n = np.floor(UR).astype(np.int64) - lo
    return lo, n


def _order(lo_b: np.ndarray, n_b: np.ndarray) -> np.ndarray:
    """Proposal emission order for one batch: ascending max-needed dq (lets
    stage 1 start while later input DMAs stream), then by (sd, nd) so
    proposals with identical d-axis bins are adjacent (stage-2 merging)."""
    dq1 = (lo_b[:, 0] + n_b[:, 0] + 3) // 4
    key = (dq1 * 32 + lo_b[:, 0]) * 32 + n_b[:, 0]
    return np.argsort(key, kind="stable")


def _mk_ap(base_ap, extra_offset: int, dims):
    """Custom free-dim AP on a tile AP: keep the partition dim, replace free
    dims with [step, count] pairs and add an element offset."""
    ap = base_ap.copy()
    ap.ap = mybir.VecI64Pair([list(base_ap.ap[0])] + [[s, c] for s, c in dims])
    ap.offset = base_ap.offset + extra_offset
    return ap


def _reduce(nc, out_ap, in_ap, nred):
    axis = mybir.AxisListType.XY if nred == 2 else mybir.AxisListType.X
    nc.vector.tensor_reduce(
        out=out_ap, in_=in_ap, axis=axis, op=mybir.AluOpType.max
    )


def _hoist_extra_waits(nc: bass.Bass) -> None:
    """This walrus encodes at most one sync-wait per instruction. Move all but
    one wait of any multi-wait instruction onto standalone EventSemaphore ops
    inserted immediately before it in the same (in-order) engine stream --
    semantically equivalent, just an earlier block point for the same sems."""
    for bb in nc.m.functions[0].blocks:
        insts = bb.instructions
        i = 0
        while i < len(insts):
            ins = insts[i]
            si = ins.sync_info
            if si is not None and len(si.on_wait) > 1 and ins.opcode != "EventSemaphore":
                waits = list(si.on_wait)
                for j, w in enumerate(waits[:-1]):
                    insts.insert(
                        i + j,
                        mybir.InstEventSemaphore(
                            name=f"{ins.name}_w{j}",
                            engine=ins.engine,
                            ins=[], outs=[],
                            sync_info=mybir.SyncInfo(on_wait=[w], on_update=[]),
                        ),
                    )
                si.on_wait = [waits[-1]]
                i += len(waits) - 1
            i += 1


def _build_program(
    lo: np.ndarray, n: np.ndarray, sim_guards: bool = False, hoist: bool = True
) -> bass.Bass:
    nc = bass.Bass("TRN2", target_bir_lowering=False, debug=False, num_devices=8)
    fm_d = nc.dram_tensor("fm_part", [CH, S, S, S], mybir.dt.float32, kind="ExternalInput")
    out_d = nc.dram_tensor("out_part", [128, 128], mybir.dt.float32, kind="ExternalOutput")

    orders = [_order(lo[b], n[b]) for b in range(B)]
    # Input DMAs ride the sync HWDGE queue; shuffle + store ride the scalar
    # queue. Each queue has ONE completion semaphore, so every TensorReduce
    # needs at most one sync-wait (this walrus rejects TRs with 2+ waits).

    with tile.TileContext(nc) as tc:
        with tc.tile_pool(name="pool", bufs=1) as pool:
            A = pool.tile([128, 6 * S * S], mybir.dt.float32)   # (d%4,c) x (dq,h,w)
            Bt = pool.tile([128, NG * GSZ], mybir.dt.float32)   # (d%4,c) x (g,hb,wb,dq,pi)
            Ct = pool.tile([128, NG * S * GP], mybir.dt.float32)  # (c,hb,wb) x (g,d,pi)
            Dt = pool.tile([128, 2 * P], mybir.dt.float32)      # (c,hb,wb) x (sp,db)

            # Zero-fill intermediates on the otherwise-idle gpsimd engine.
            # Required for interpreter-backed execution paths (their init
            # tracking rejects reads over tiles with interleaved partial
            # writes; the never-written dq slots are dead data on hardware).
            # The extra sync-waits this puts on the first reduces are split
            # off by _hoist_extra_waits below.
            # Load the core id first: the If dispatch on every engine gates on
            # this DRAM->register load, so start its latency at t=0.
            pid = nc.partition_id(
                engines=(mybir.EngineType.DVE, mybir.EngineType.Pool)
            )
            arm = pid >> 1

            # Per-group slices so stage1(g0) isn't gated on zeroing the
            # whole tile.
            for g in range(NG):
                nc.gpsimd.memset(_mk_ap(Bt[:], g * GSZ, [[1, GSZ]]), 0)
            nc.gpsimd.memset(Dt[:], 0)
            for g in range(NG):
                nc.gpsimd.memset(_mk_ap(Ct[:], g * S * GP, [[1, S * GP]]), 0)

            # ---- input: fm [32,24,24,24] -> A; two DMAs per dq (128 parts
            # each, half the h-range) so the first reduces unblock sooner ----
            for dq in range(6):
                for hh in range(2):
                    src = fm_d.ap().copy()
                    src.ap = mybir.VecI64Pair(
                        [[S * S, 4], [S * S * S, CH], [1, S * S // 2]]
                    )
                    src.offset = dq * 4 * S * S + hh * (S * S // 2)
                    (nc.sync if dq % 2 == 0 else nc.scalar).dma_start(
                        _mk_ap(A[:], dq * S * S + hh * (S * S // 2),
                               [[1, S * S // 2]]),
                        src,
                    )


            # Arms hold ONLY DVE reduces (Tile's If balances engine sems at
            # reconverge, but not DMA-lane sems, so DMAs live outside arms).
            # Group-interleaved emission pipelines stage1(g+1) with
            # shuffle(g)/stage2(g)/store(g).
            def stage1(b, g):
                for pi in range(GP):
                    sp = g * GP + pi
                    p = int(orders[b][sp])
                    nd, nh, nw = (int(n[b, p, a]) for a in range(3))
                    sd, sh, sw = (int(lo[b, p, a]) for a in range(3))
                    lh, lw = (nh + 1) // 2, (nw + 1) // 2
                    dq0 = sd // 4
                    ndq = (sd + nd + 3) // 4 - dq0
                    red = ([[S, lh]] if lh > 1 else []) + (
                        [[1, lw]] if lw > 1 else []
                    )
                    if not red:
                        # lh == lw == 1: stage 1 is a pure strided copy; run
                        # it on the otherwise-idle gpsimd engine instead of
                        # spending a DVE sequencer slot. (Engine sems inside
                        # tc.If arms are balanced at reconverge, so non-DVE
                        # compute is safe here -- unlike DMAs.)
                        din = [[(nh // 2) * S, 2], [nw // 2, 2]]
                        dout = [[2 * 6 * GP, 2], [6 * GP, 2]]
                        if ndq > 1:
                            din.insert(0, [576, ndq])
                            dout.insert(0, [GP, ndq])
                        nc.gpsimd.tensor_copy(
                            _mk_ap(Bt[:], g * GSZ + dq0 * GP + pi, dout),
                            _mk_ap(A[:], dq0 * 576 + sh * S + sw, din),
                        )
                        continue
                    nred = len(red)
                    if (1 if ndq > 1 else 0) + 2 + len(red) <= 4:
                        # HW APs are <= 5-D incl. partition dim: merge both
                        # h-bins into one reduce when that fits
                        din = [[(nh // 2) * S, 2], [nw // 2, 2]]
                        dout = [[2 * 6 * GP, 2], [6 * GP, 2]]
                        if ndq > 1:
                            din.insert(0, [576, ndq])
                            dout.insert(0, [GP, ndq])
                        in_ap = _mk_ap(A[:], dq0 * 576 + sh * S + sw, din + red)
                        out_ap = _mk_ap(Bt[:], g * GSZ + dq0 * GP + pi, dout)
                        _reduce(nc, out_ap, in_ap, nred)
                    else:
                        for hb in range(2):
                            in_ap = _mk_ap(
                                A[:],
                                dq0 * 576 + (sh + hb * (nh // 2)) * S + sw,
                                [[576, ndq], [nw // 2, 2]] + red,
                            )
                            out_ap = _mk_ap(
                                Bt[:],
                                g * GSZ + hb * (2 * 6 * GP) + dq0 * GP + pi,
                                [[GP, ndq], [6 * GP, 2]],
                            )
                            _reduce(nc, out_ap, in_ap, nred)

            def stage2(b, g):
                # proposals ordered so identical (sd, nd) runs are adjacent:
                # one reduce per run (leading pi dim)
                sp = g * GP
                while sp < (g + 1) * GP:
                    p = int(orders[b][sp])
                    pi = sp % GP
                    nd = int(n[b, p, 0])
                    sd = int(lo[b, p, 0])
                    ld = (nd + 1) // 2
                    m = 1
                    while (
                        sp + m < (g + 1) * GP
                        and int(n[b, orders[b][sp + m], 0]) == nd
                        and int(lo[b, orders[b][sp + m], 0]) == sd
                    ):
                        m += 1
                    out_ap = _mk_ap(Dt[:], sp * 2, [[2, m], [1, 2]])
                    if ld == 1:
                        # nd == 2: both d-bins are single slices -> pure copy;
                        # run on the idle gpsimd engine (1-input Pool ops are
                        # walrus-encodable, tensor_tensor is not)
                        nc.gpsimd.tensor_copy(
                            out_ap,
                            _mk_ap(
                                Ct[:], g * (S * GP) + sd * GP + pi,
                                [[1, m], [(nd // 2) * GP, 2]],
                            ),
                        )
                    else:
                        in_ap = _mk_ap(
                            Ct[:], g * (S * GP) + sd * GP + pi,
                            [[1, m], [(nd // 2) * GP, 2], [GP, ld]],
                        )
                        _reduce(nc, out_ap, in_ap, 1)
                    sp += m

            # Software pipeline with lag 2: visit v runs stage1(v) and
            # stage2(v-2), so each stage-2 group's shuffle (incl. its DMA
            # completion latency) hides under a full stage-1 group.
            LAG = 3
            for v in range(NG + LAG):
                for b in range(B):
                    with tc.If(arm == b):
                        if v < NG:
                            stage1(b, v)
                        if v >= LAG:
                            stage2(b, v - LAG)
                if v < NG:
                    # shuffle group v: Bt -> Ct, per dm4, split over both
                    # HWDGE queues (the wait-hoist pass splits any resulting
                    # multi-wait consumers)
                    for dm4 in range(4):
                        s_ap = _mk_ap(
                            Bt[dm4 * 32:(dm4 + 1) * 32], v * GSZ, [[1, GSZ]]
                        )
                        d_ap = _mk_ap(
                            Ct[:], v * (S * GP) + dm4 * GP,
                            [[4 * GP, 6], [1, GP]],
                        )
                        (nc.scalar if dm4 % 2 == 0 else nc.sync).dma_start(
                            d_ap, s_ap
                        )
                if v == NG + LAG - 2:
                    # store groups 0..NG-2 while the last stage-2 group runs
                    half = (NG - 1) * 2 * GP
                    nc.sync.dma_start(
                        _mk_ap(out_d.ap(), 0, [[1, half]]),
                        _mk_ap(Dt[:], 0, [[1, half]]),
                    )
            last = (NG - 1) * 2 * GP
            nc.scalar.dma_start(
                _mk_ap(out_d.ap(), last, [[1, 2 * P - last]]),
                _mk_ap(Dt[:], last, [[1, 2 * P - last]]),
            )

    if hoist:
        # CoreSim can't execute the hoisted EventSemaphores (they lack its
        # scheduler-added fake updates); walrus birsim validates them instead.
        _hoist_extra_waits(nc)
    return nc


def _run(fm: np.ndarray, corners: np.ndarray, trace: bool = False, trace_cores=None):
    fm = np.asarray(fm, dtype=np.float32)
    corners = np.asarray(corners, dtype=np.float32)
    assert fm.shape == (B, C, S, S, S) and corners.shape == (B, P, 2, 3)

    lo, n = _bin_params(corners)
    nc = _build_program(lo, n)

    in_maps = []
    for k in range(8):
        b, ch = k // 2, k % 2
        in_maps.append(
            {"fm_part": np.ascontiguousarray(fm[b, ch * CH:(ch + 1) * CH])}
        )

    res = run_bass_kernel_spmd(
        nc, in_maps, core_ids=list(range(8)), trace=trace,
        **({"trace_cores": trace_cores} if trace_cores else {}),
    )

    out = np.empty((B, P, C, 2, 2, 2), dtype=np.float32)
    for k in range(8):
        b, ch = k // 2, k % 2
        r = res.results[k]["out_part"]  # [128, 128] = [(c,kh,kw), (sp,kd)]
        r = r.reshape(CH, 2, 2, P, 2).transpose(3, 0, 4, 1, 2)  # [sp,c,kd,kh,kw]
        out[b, _order(lo[b], n[b]), ch * CH:(ch + 1) * CH] = r
    return out, res


def kernel(fm: np.ndarray, corners: np.ndarray) -> np.ndarray:
    out, _ = _run(fm, corners)
    return out

